# revision 10
# baseline (speedup 1.0000x reference)
"""Causal self-attention Trainium2 kernel.

Problem: B=8, T=1024, C=768, H=12 heads, D=64. fp32.
Strategy: pure data-parallel over batch — each of the 8 NeuronCores computes
one batch element's full attention block, fully fused on-chip (qkv matmul,
causal softmax without max-subtraction, attention, output projection).

Layout choices (to avoid any on-device transposes):
  - x is transposed host-side to xT [C, T].
  - Q^T, K^T are computed in transposed layout [qkv_col, T] with w_attn tiles
    as the stationary operand and xT as the moving operand.
  - V is computed in natural layout [T, col] with xT tiles stationary.
  - Attention scores are computed directly transposed: weiT[s, t] via
    lhsT=k^T, rhs=q^T. Softmax = exp(weiT)/S (no max subtraction, safe in
    fp32 for this data distribution); exp runs on ScalarE PSUM->SBUF.
  - p@v uses stationary [v | ones] so PSUM row 64 accumulates the softmax
    denominators S[t] for free; normalization commutes to a single
    VectorE multiply per head during the PSUM->SBUF move.
  - Projection uses att_out^T tiles stationary, w_proj moving -> y natural.
"""

import numpy as np
from contextlib import ExitStack

import bass_rust
import concourse.bass as bass
import concourse.tile as tile
from concourse import mybir
from concourse.bass_utils import run_bass_kernel_spmd

F32 = mybir.dt.float32
AF = mybir.ActivationFunctionType

B, T, C = 8, 1024, 768
H, D = 12, 64
NT = T // 128       # 8 token tiles
KC = C // 128       # 6 contraction chunks
MQK = 2 * C // 128  # 12 m-tiles covering q,k columns (0..1535)
NEG = -1.0e30


def _patched_drain_and_barrier(self, tick_clock, wait_clock):
    # Walrus in this environment rejects >1 sync-wait on a single SP drain
    # ("Too many sync wait commands"); split the tail waits across a chain
    # of drains carrying one wait each.
    nc_ = self.nc
    drain_inst = nc_.sync.drain()
    wait_clock.add_sem_waits(
        drain_inst.ins, bass_rust.ScopedClock({None: tick_clock.global_clock})
    )
    si = drain_inst.ins.sync_info
    waits = list(si.on_wait or [])
    if len(waits) > 1:
        si.on_wait = waits[:1]
        for i in range(1, len(waits)):
            extra = nc_.sync.drain()
            extra.ins.sync_info = bass_rust.SyncInfo(
                on_wait=waits[i : i + 1], on_update=[]
            )
    nc_.all_engine_barrier()
    popped = nc_._tile_sem_poison_stack.pop()
    assert popped is self._sem_poison
    nc_.clear_and_free_semaphores(list(self.sems.allocated().values()))
    nc_.all_engine_barrier()


tile.TileContext._drain_and_barrier = _patched_drain_and_barrier


def _split_multi_waits(nc, max_waits=1):
    """Walrus here allows only `max_waits` sync-wait commands per instruction.
    Hoist excess waits onto standalone EventSemaphore ops inserted just before
    the owning instruction on the same engine (same blocking semantics)."""
    n_new = 0
    for fn in nc.m.functions:
        for blk in fn.blocks:
            insts = blk.instructions
            out = []
            for inst in insts:
                si = getattr(inst, "sync_info", None)
                waits = list(si.on_wait) if si and si.on_wait else []
                if len(waits) > max_waits:
                    keep = waits[-max_waits:]
                    hoist = waits[: -max_waits]
                    for w in hoist:
                        ev = mybir.InstEventSemaphore(
                            name=f"Wsplit-{nc.next_id()}", ins=[], outs=[]
                        )
                        ev.engine = inst.engine
                        ev.sync_info = bass_rust.SyncInfo(
                            on_wait=[w], on_update=[]
                        )
                        nc.inst_map[ev.name] = ev
                        out.append(ev)
                        n_new += 1
                    si.on_wait = keep
                out.append(inst)
            if n_new:
                insts[:] = out
    return n_new


def _t_segments(t_lo):
    """Split [t_lo, 1024) into matmul-legal (<=512, bank-aligned) segments."""
    segs = []
    if t_lo < 512:
        segs.append((t_lo, 512))
        segs.append((512, 1024))
    else:
        segs.append((t_lo, 1024))
    return segs


def build_attention_kernel():
    nc = bass.Bass("TRN2", target_bir_lowering=False, debug=False)

    xT = nc.dram_tensor("xT", [C, T], F32, kind="ExternalInput").ap()
    wa = nc.dram_tensor("wa", [C, 3 * C], F32, kind="ExternalInput").ap()
    baqk = nc.dram_tensor("baqk", [128, MQK], F32, kind="ExternalInput").ap()
    bv = nc.dram_tensor("bv", [128, C], F32, kind="ExternalInput").ap()
    wp = nc.dram_tensor("wp", [C, C], F32, kind="ExternalInput").ap()
    bp = nc.dram_tensor("bp", [128, C], F32, kind="ExternalInput").ap()
    mask = nc.dram_tensor("mask", [128, 128], F32, kind="ExternalInput").ap()
    y = nc.dram_tensor("y", [T, C], F32, kind="ExternalOutput").ap()

    with tile.TileContext(nc) as tc, ExitStack() as ctx:
        consts = ctx.enter_context(tc.tile_pool(name="consts", bufs=1))
        qk_pool = ctx.enter_context(tc.tile_pool(name="qkT", bufs=1))
        v_pool = ctx.enter_context(tc.tile_pool(name="vsb", bufs=1))

        baqk_sb = consts.tile([128, MQK], F32)
        nc.sync.dma_start(baqk_sb[:], baqk[:])
        bv_sb = consts.tile([128, C], F32)
        nc.sync.dma_start(bv_sb[:], bv[:])
        bp_sb = consts.tile([128, C], F32)
        nc.sync.dma_start(bp_sb[:], bp[:])
        mask_sb = consts.tile([128, 128], F32)
        nc.sync.dma_start(mask_sb[:], mask[:])
        ones_sb = consts.tile([128, 64], F32)
        nc.vector.memset(ones_sb[:], 1.0)
        wp_sb = []
        for kc in range(KC):
            t = consts.tile([128, C], F32, tag=f"wp{kc}", name=f"wp_sb{kc}")
            nc.sync.dma_start(t[:], wp[kc * 128 : (kc + 1) * 128, :])
            wp_sb.append(t)

        # Q^T,K^T: tile m holds qkv columns [m*128,(m+1)*128) over all T.
        qkT = []
        for m in range(MQK):
            qkT.append(qk_pool.tile([128, T], F32, tag=f"qkT{m}", name=f"qkT{m}"))
        # V + ones column: per (t_tile, head) 65 columns: [v(64) | 1].
        v_sb = v_pool.tile([128, NT, H, 65], F32)
        nc.vector.memset(v_sb[:, :, :, 64], 1.0)

        # ---- Phase A/B: qkv projections ----
        with (
            tc.tile_pool(name="loads", bufs=1) as loads,
            tc.tile_pool(name="qkv_psum", bufs=2, space="PSUM") as qkv_psum,
        ):
            xT_sb = []
            wa_sb = []
            for kc in range(KC):
                xt = loads.tile([128, T], F32, tag=f"xT{kc}", name=f"xT_sb{kc}")
                nc.sync.dma_start(xt[:], xT[kc * 128 : (kc + 1) * 128, :])
                xT_sb.append(xt)
                wt = loads.tile([128, 3 * C], F32, tag=f"wa{kc}", name=f"wa_sb{kc}")
                nc.sync.dma_start(wt[:], wa[kc * 128 : (kc + 1) * 128, :])
                wa_sb.append(wt)

            # Q^T / K^T m-tiles: stationary = w_attn tile, moving = xT.
            for m in range(MQK):
                qk_ps = qkv_psum.tile([128, T], F32, tag="qk")
                for kc in range(KC):
                    lhsT = wa_sb[kc][:, m * 128 : (m + 1) * 128]
                    for nb in range(2):
                        nc.tensor.matmul(
                            qk_ps[:, nb * 512 : (nb + 1) * 512],
                            lhsT,
                            xT_sb[kc][:, nb * 512 : (nb + 1) * 512],
                            start=(kc == 0),
                            stop=(kc == KC - 1),
                        )
                nc.scalar.activation(
                    qkT[m][:], qk_ps[:], AF.Identity, bias=baqk_sb[:, m : m + 1]
                )

            # V t-tiles: stationary = xT tile, moving = w_attn[:, 1536:2304].
            for tt in range(NT):
                v_ps = qkv_psum.tile([128, C], F32, tag="v")
                for kc in range(KC):
                    lhsT = xT_sb[kc][:, tt * 128 : (tt + 1) * 128]
                    nc.tensor.matmul(
                        v_ps[:, 0:512],
                        lhsT,
                        wa_sb[kc][:, 2 * C : 2 * C + 512],
                        start=(kc == 0),
                        stop=(kc == KC - 1),
                    )
                    nc.tensor.matmul(
                        v_ps[:, 512:768],
                        lhsT,
                        wa_sb[kc][:, 2 * C + 512 : 3 * C],
                        start=(kc == 0),
                        stop=(kc == KC - 1),
                    )
                nc.vector.tensor_add(
                    v_sb[:, tt, :, 0:64],
                    v_ps.rearrange("p (h d) -> p h d", h=H),
                    bv_sb.rearrange("p (h d) -> p h d", h=H),
                )

        # ---- Phase C: attention per head;  Phase D: projection ----
        with tc.tile_pool(name="attT", bufs=1) as attT_pool:
            attT = []
            for kc in range(KC):
                attT.append(attT_pool.tile([128, T], F32, tag=f"attT{kc}", name=f"attT{kc}"))

            with (
                tc.tile_pool(name="attn_work", bufs=3) as work,
                tc.tile_pool(name="recs", bufs=2) as recs,
                tc.tile_pool(name="attn_psum", bufs=2, space="PSUM") as attn_psum,
            ):
                for h in range(H):
                    par = h % 2
                    mq = h // 2
                    q_ap = qkT[mq][par * 64 : (par + 1) * 64, :]
                    k_ap = qkT[MQK // 2 + mq][par * 64 : (par + 1) * 64, :]

                    outT_ps = attn_psum.tile([128, T], F32, tag="outT")
                    for j in range(NT):
                        t_lo = j * 128
                        segs = _t_segments(t_lo)
                        wei_ps = attn_psum.tile([128, T], F32, tag="wei")
                        kl = k_ap[:, j * 128 : (j + 1) * 128]
                        for a, b in segs:
                            nc.tensor.matmul(
                                wei_ps[:, a:b], kl, q_ap[:, a:b],
                                start=True, stop=True,
                            )
                        # causal mask on the diagonal 128x128 chunk
                        nc.vector.tensor_add(
                            wei_ps[:, t_lo : t_lo + 128],
                            wei_ps[:, t_lo : t_lo + 128],
                            mask_sb[:],
                        )
                        pT = work.tile([128, T], F32, tag="pT")
                        nc.scalar.activation(
                            pT[:, t_lo:T], wei_ps[:, t_lo:T], AF.Exp
                        )
                        vl = v_sb[:, j, h, 0:65]
                        for a, b in segs:
                            # per 512-half: j==0 initializes the full half,
                            # later j accumulate partial ranges; last writer
                            # of half0 is j==3, of half1 is j==7.
                            nc.tensor.matmul(
                                outT_ps[0:65, a:b], vl, pT[:, a:b],
                                start=(j == 0),
                                stop=(j == NT - 1) or (b == 512 and j == 3),
                                skip_group_check=True,
                            )

                    # normalize: att_out^T_h = psum rows 0..63 * (1/S) with
                    # S = psum row 64; place at attT chunk rows par*64..
                    recS = recs.tile([128, T], F32, tag="recS")
                    nc.vector.reciprocal(recS[64:65, :], outT_ps[64:65, :])
                    # broadcast 1/S across 64 partitions via a rank-1 matmul:
                    # lhsT = ones [1,64] (at partition 64, matching recS row)
                    recB_ps = attn_psum.tile([128, T], F32, tag="wei")
                    for nb in range(2):
                        nc.tensor.matmul(
                            recB_ps[0:64, nb * 512 : (nb + 1) * 512],
                            ones_sb[64:65, :],
                            recS[64:65, nb * 512 : (nb + 1) * 512],
                            start=True,
                            stop=True,
                        )
                    # VE can read only one PSUM operand; stage recB in SBUF.
                    recB = recs.tile([128, T], F32, tag="recB")
                    nc.scalar.copy(recB[0:64, :], recB_ps[0:64, :])
                    if par == 0:
                        nc.vector.tensor_mul(
                            attT[mq][0:64, :], outT_ps[0:64, :], recB[0:64, :]
                        )
                    else:
                        # VE is lane-locked; normalize at rows 0..63 then DMA
                        # the partition shift into attT rows 64..127.
                        shift = work.tile([128, T], F32, tag="shift")
                        nc.vector.tensor_mul(
                            shift[0:64, :], outT_ps[0:64, :], recB[0:64, :]
                        )
                        nc.sync.dma_start(attT[mq][64:128, :], shift[0:64, :])

            # ---- Phase D: projection ----
            with (
                tc.tile_pool(name="proj_out", bufs=3) as proj_out,
                tc.tile_pool(name="proj_psum", bufs=2, space="PSUM") as proj_psum,
            ):
                for tt in range(NT):
                    y_ps = proj_psum.tile([128, C], F32, tag="y")
                    for kc in range(KC):
                        lhsT = attT[kc][:, tt * 128 : (tt + 1) * 128]
                        nc.tensor.matmul(
                            y_ps[:, 0:512], lhsT, wp_sb[kc][:, 0:512],
                            start=(kc == 0), stop=(kc == KC - 1),
                        )
                        nc.tensor.matmul(
                            y_ps[:, 512:768], lhsT, wp_sb[kc][:, 512:768],
                            start=(kc == 0), stop=(kc == KC - 1),
                        )
                    y_sb = proj_out.tile([128, C], F32, tag="ysb")
                    nc.vector.tensor_add(y_sb[:], y_ps[:], bp_sb[:])
                    nc.sync.dma_start(y[tt * 128 : (tt + 1) * 128, :], y_sb[:])

    _split_multi_waits(nc)
    return nc


_NC_CACHE = None


def _get_nc():
    global _NC_CACHE
    if _NC_CACHE is None:
        _NC_CACHE = build_attention_kernel()
    return _NC_CACHE


def make_in_maps(x, w_attn, b_attn, w_proj, b_proj):
    x = np.asarray(x, dtype=np.float32)
    w_attn = np.ascontiguousarray(np.asarray(w_attn, dtype=np.float32))
    b_attn = np.asarray(b_attn, dtype=np.float32)
    w_proj = np.ascontiguousarray(np.asarray(w_proj, dtype=np.float32))
    b_proj = np.asarray(b_proj, dtype=np.float32)

    baqk = np.ascontiguousarray(b_attn[: 2 * C].reshape(MQK, 128).T)
    bv = np.ascontiguousarray(np.broadcast_to(b_attn[2 * C :], (128, C)))
    bp = np.ascontiguousarray(np.broadcast_to(b_proj, (128, C)))
    sl, tl = np.meshgrid(np.arange(128), np.arange(128), indexing="ij")
    mask = np.where(tl >= sl, 0.0, NEG).astype(np.float32)

    in_maps = []
    for b in range(B):
        in_maps.append(
            {
                "xT": np.ascontiguousarray(x[b].T),
                "wa": w_attn,
                "baqk": baqk,
                "bv": bv,
                "wp": w_proj,
                "bp": bp,
                "mask": mask,
            }
        )
    return in_maps


def kernel(x, w_attn, b_attn, w_proj, b_proj):
    nc = _get_nc()
    in_maps = make_in_maps(x, w_attn, b_attn, w_proj, b_proj)
    res = run_bass_kernel_spmd(nc, in_maps, core_ids=list(range(B)))
    return np.stack([res.results[i]["y"] for i in range(B)], axis=0)
